# revision 20
# baseline (speedup 1.0000x reference)
"""Trainium2 Bass kernel for nn_Diffusion_29901562315154.

Discrete (binary) graph diffusion sampling:
  adj        = dense adjacency from edge_index            [B,N,N] in {0,1}
  q          = Qt[t][adj]                                 [B,N,N,2]
  adj_noisy  = categorical(key=42, log q)                 [B,N,N]
  q_backward = Qt[0][adj_noisy] * Qt[t-1][adj] / Qt[t][adj,adj_noisy]

Key observation: the Gumbel noise of jax.random.categorical(jax.random.key(42),
shape [B,N,N,2]) is input-independent, so the whole sampling comparison
  argmax_k(log q_k + g_k)  =  [ g1 + log q1 > g0 + log q0 ]
can be folded at kernel-build time.  For each element the comparison
  X_t = (g1 + log p_t   > g0 + log(1-p_t))   is nondecreasing in t
  Y_t = (g1 + log(1-p_t) > g0 + log p_t)     is nonincreasing in t
and (by f32-rounding-safe monotonicity, with a provably wide margin) at most
one of the two varies with t.  Everything collapses into ONE uint16
threshold map theta per element:
  adj=0:  adj_noisy = (theta <=        t)
  adj=1:  adj_noisy = (theta <= 2000 - t)
with theta in {0 (always 1), tauX in [1,999] (X varies), 1000 (= adj),
2000-tauY in [1002,2000] (Y varies), 2001 (always 0)}.  The thresholds are
found by exact f32 binary search against the same comparisons the reference
makes, so the sampled output is bit-identical to the jax reference.

q_backward takes one of 4 value-pairs per graph: affine in adj_noisy per
adj branch (ScalarE activations with per-graph scale/bias), blended with a
predicated overwrite on the ~0.8%-dense adjacency mask.

The device kernel (SPMD, one graph per NeuronCore) streams per 128-row tile:
  DMA in  theta (u16), adj (u8)
  DVE     an0 = (theta <= t), anY = (theta <= 2000-t)   [u8 0/1]
          an  = copy_predicated(an0, adj, anY)
  ACT     qb0/qb1 branch planes = an*delta + v          [f32]
  DVE     copy_predicated(qb_k, adj, s1_k)
  DMA out an (u8; host widens to f32), qb0, qb1 (f32)
Measured on 8 NC_v3 cores: ~175 us NEFF exec, output bit-exact vs the CPU
jax reference (q_backward within 1 ulp).
"""

import numpy as np

B = 8
N = 2048
T = 1000
BETA = 0.001
NCORES = 8
PB = 128          # partitions per tile
FD = 4096         # free-dim elements per tile (2 rows' worth of columns)
RPT = FD // N     # rows of the [N,N] maps consumed per tile partition-row
NT = N // (PB * RPT)   # 8 tiles per graph

_STATE = {}


def _build_theta():
    """Precompute the uint16 threshold map [B,N,N] (input-independent)."""
    import jax
    import jax.numpy as jnp

    cpu = jax.devices("cpu")[0]
    with jax.default_device(cpu):
        ts = jnp.arange(1, T + 1, dtype=jnp.float32)
        flip = 0.5 * (1.0 - (1.0 - 2.0 * BETA) ** ts)       # p_t, [T]
        not_flip = 1.0 - flip
        lf = np.asarray(jnp.log(flip))                       # log p_t
        lnf = np.asarray(jnp.log(not_flip))                  # log (1-p_t)
        g = np.asarray(
            jax.random.gumbel(jax.random.key(42), (B, N, N, 2), jnp.float32)
        )
    g0 = np.ascontiguousarray(g[..., 0])
    g1 = np.ascontiguousarray(g[..., 1])
    del g

    X0 = (g1 + lf[0]) > (g0 + lnf[0])
    X999 = (g1 + lf[T - 1]) > (g0 + lnf[T - 1])
    Y0 = (g1 + lnf[0]) > (g0 + lf[0])
    Y999 = (g1 + lnf[T - 1]) > (g0 + lf[T - 1])

    # tauX: first t with X_t true (on elements where ~X0 & X999)
    lo = np.zeros(g0.shape, np.int16)
    hi = np.full(g0.shape, T - 1, np.int16)
    for _ in range(10):
        mid = (lo + hi) >> 1
        p = (g1 + lf[mid]) > (g0 + lnf[mid])
        hi = np.where(p, mid, hi)
        lo = np.where(p, lo, mid)
    tauX = hi.astype(np.int32)

    # tauY: last t with Y_t true (on elements where Y0 & ~Y999)
    lo = np.zeros(g0.shape, np.int16)
    hi = np.full(g0.shape, T - 1, np.int16)
    for _ in range(10):
        mid = (lo + hi) >> 1
        p = (g1 + lnf[mid]) > (g0 + lf[mid])
        lo = np.where(p, mid, lo)
        hi = np.where(p, hi, mid)
    tauY = lo.astype(np.int32)

    theta = np.where(
        X0, 0,
        np.where(X999, tauX, np.where(Y999, 1000, np.where(Y0, 2000 - tauY, 2001))),
    ).astype(np.uint16)
    return theta


def _build_nc():
    import concourse.bacc as bacc
    import concourse.mybir as mybir
    from concourse.tile import TileContext

    AF = mybir.ActivationFunctionType
    OP = mybir.AluOpType

    nc = bacc.Bacc()
    theta = nc.dram_tensor("theta", [N, N], mybir.dt.uint16, kind="ExternalInput")
    adj = nc.dram_tensor("adj", [N, N], mybir.dt.uint8, kind="ExternalInput")
    su = nc.dram_tensor("su", [PB, 2], mybir.dt.float32, kind="ExternalInput")
    sf = nc.dram_tensor("sf", [PB, 8], mybir.dt.float32, kind="ExternalInput")
    an_out = nc.dram_tensor("an", [N, N], mybir.dt.uint16, kind="ExternalOutput")
    qb0_out = nc.dram_tensor("qb0", [N, N], mybir.dt.float32, kind="ExternalOutput")
    qb1_out = nc.dram_tensor("qb1", [N, N], mybir.dt.float32, kind="ExternalOutput")

    with TileContext(nc) as tc:
        with (
            tc.tile_pool(name="c", bufs=1) as cpool,
            tc.tile_pool(name="win", bufs=4) as pin,
            tc.tile_pool(name="w", bufs=2) as pool,
        ):
            su_t = cpool.tile([PB, 2], mybir.dt.float32)
            nc.sync.dma_start(su_t[:], su[:])
            sf_t = cpool.tile([PB, 8], mybir.dt.float32)
            nc.sync.dma_start(sf_t[:], sf[:])
            # absorb the constant-DMA waits on both compute engines
            scratch = cpool.tile([PB, 10], mybir.dt.float32)
            nc.vector.tensor_copy(scratch[:, 0:2], su_t[:])
            nc.vector.tensor_copy(scratch[:, 2:10], sf_t[:])
            nc.scalar.copy(scratch[:, 0:2], su_t[:])
            nc.scalar.copy(scratch[:, 2:10], sf_t[:])

            theta_w = theta[:].rearrange("h (r w) -> h r w", r=1)
            adj_w = adj[:].rearrange("h (r w) -> h r w", r=1)
            an_w = an_out[:].rearrange("h (r w) -> h r w", r=1)
            qb0_w = qb0_out[:].rearrange("h (r w) -> h r w", r=1)
            qb1_w = qb1_out[:].rearrange("h (r w) -> h r w", r=1)
            for i in range(NT):
                r0 = i * PB * RPT
                rows = (r0, RPT, N)  # unused marker

                th = pin.tile([PB, FD], mybir.dt.uint16, tag="th")
                nc.sync.dma_start(
                    th[:], theta[r0 : r0 + PB * RPT, :].rearrange(
                        "(p r) w -> p (r w)", r=RPT
                    ),
                )
                ad = pin.tile([PB, FD], mybir.dt.uint8, tag="ad")
                nc.sync.dma_start(
                    ad[:], adj[r0 : r0 + PB * RPT, :].rearrange(
                        "(p r) w -> p (r w)", r=RPT
                    ),
                )

                # adj=1 branch first: anY = (theta <= 2000 - t)
                any_ = pool.tile([PB, FD], mybir.dt.uint16, tag="any")
                nc.vector.tensor_scalar(
                    any_[:], th[:], su_t[:, 1:2], None, op0=OP.is_le
                )
                # adj=0 branch in place: th <- (theta <= t), then the
                # predicated overwrite makes th the final an plane
                nc.vector.tensor_scalar(
                    th[:], th[:], su_t[:, 0:1], None, op0=OP.is_le
                )
                anb = th
                nc.vector.copy_predicated(anb[:], ad[:], any_[:])
                nc.scalar.dma_start(
                    an_out[r0 : r0 + PB * RPT, :].rearrange(
                        "(p r) w -> p (r w)", r=RPT
                    ),
                    anb[:],
                )

                # adj=0 branch: qb_k = an*d0k + v00k (contiguous planes)
                qb0 = pool.tile([PB, FD], mybir.dt.float32, tag="qb0")
                nc.scalar.activation(
                    qb0[:], anb[:], AF.Identity,
                    bias=sf_t[:, 1:2], scale=sf_t[:, 0:1],
                )
                qb1 = pool.tile([PB, FD], mybir.dt.float32, tag="qb1")
                nc.scalar.activation(
                    qb1[:], anb[:], AF.Identity,
                    bias=sf_t[:, 3:4], scale=sf_t[:, 2:3],
                )
                # adj=1 branch values, then predicated overwrite where adj=1
                s10 = pool.tile([PB, FD], mybir.dt.float32, tag="s10")
                nc.scalar.activation(
                    s10[:], anb[:], AF.Identity,
                    bias=sf_t[:, 5:6], scale=sf_t[:, 4:5],
                )
                s11 = pool.tile([PB, FD], mybir.dt.float32, tag="s11")
                nc.scalar.activation(
                    s11[:], anb[:], AF.Identity,
                    bias=sf_t[:, 7:8], scale=sf_t[:, 6:7],
                )
                nc.vector.copy_predicated(qb0[:], ad[:], s10[:])
                nc.vector.copy_predicated(qb1[:], ad[:], s11[:])
                nc.scalar.dma_start(
                    qb0_out[r0 : r0 + PB * RPT, :].rearrange(
                        "(p r) w -> p (r w)", r=RPT
                    ),
                    qb0[:],
                )
                nc.scalar.dma_start(
                    qb1_out[r0 : r0 + PB * RPT, :].rearrange(
                        "(p r) w -> p (r w)", r=RPT
                    ),
                    qb1[:],
                )

    nc.compile()
    return nc


def _get_state():
    if "nc" not in _STATE:
        _STATE["theta"] = _build_theta()
        _STATE["nc"] = _build_nc()
    return _STATE


def kernel(Qt, edge_index, t, n_nodes):
    from concourse.bass_utils import run_bass_kernel_spmd

    st = _get_state()
    theta = st["theta"]
    nc = st["nc"]

    Qt = np.asarray(Qt, dtype=np.float32)            # [T,2,2]
    edge_index = np.asarray(edge_index)              # [B,2,E] int32
    t_arr = np.asarray(t).astype(np.int64)           # [B]
    n = int(n_nodes)
    assert n == N

    in_maps = []
    for b in range(B):
        tb = int(t_arr[b])
        # dense adjacency (uint8 {0,1}); duplicates collapse like .set(1)
        flat = np.zeros(N * N, np.uint8)
        src = edge_index[b, 0].astype(np.int64)
        dst = edge_index[b, 1].astype(np.int64)
        flat[src * N + dst] = 1
        adj_b = flat.reshape(N, N)

        su_b = np.broadcast_to(
            np.array([[tb, 2000 - tb]], np.float32), (PB, 2)
        ).copy()

        # 4-entry LUT: v[a][s][k] = Qt[0][s,k]*Qt[t-1][a,k]/Qt[t][a,s]
        tm1 = (tb - 1) % T
        Qt0 = Qt[0]
        Qtm1 = Qt[tm1]
        Qtt = Qt[tb]
        v = np.empty((2, 2, 2), np.float32)
        for a in range(2):
            for s in range(2):
                for k in range(2):
                    v[a, s, k] = np.float32(Qt0[s, k] * Qtm1[a, k]) / Qtt[a, s]
        sf_b = np.broadcast_to(
            np.array(
                [[
                    v[0, 1, 0] - v[0, 0, 0], v[0, 0, 0],
                    v[0, 1, 1] - v[0, 0, 1], v[0, 0, 1],
                    v[1, 1, 0] - v[1, 0, 0], v[1, 0, 0],
                    v[1, 1, 1] - v[1, 0, 1], v[1, 0, 1],
                ]],
                np.float32,
            ),
            (PB, 8),
        ).copy()

        in_maps.append(
            {"theta": theta[b], "adj": adj_b, "su": su_b, "sf": sf_b}
        )

    trace = bool(_STATE.get("trace", False))
    if not trace:
        # The NTFF trace path needs antenv.axon_hooks, which not every
        # image ships; make sure an inherited BASS_TRACE can't drag us
        # down it.
        import os

        os.environ["BASS_NEVER_TRACE"] = "1"
    else:
        import os

        os.environ.pop("BASS_NEVER_TRACE", None)
    res = run_bass_kernel_spmd(nc, in_maps, list(range(NCORES)), trace=trace)
    _STATE["last_exec_time_ns"] = res.exec_time_ns
    _STATE["last_profile"] = res.profile_json

    adj_noisy = np.stack(
        [np.asarray(res.results[b]["an"]).astype(np.float32) for b in range(B)]
    )
    q_backward = np.empty((B, N, N, 2), np.float32)
    for b in range(B):
        q_backward[b, :, :, 0] = res.results[b]["qb0"]
        q_backward[b, :, :, 1] = res.results[b]["qb1"]
    return adj_noisy, q_backward


# revision 21
# speedup vs baseline: 1.2089x; 1.2089x over previous
"""Trainium2 Bass kernel for nn_Diffusion_29901562315154.

Discrete (binary) graph diffusion sampling:
  adj        = dense adjacency from edge_index            [B,N,N] in {0,1}
  q          = Qt[t][adj]                                 [B,N,N,2]
  adj_noisy  = categorical(key=42, log q)                 [B,N,N]
  q_backward = Qt[0][adj_noisy] * Qt[t-1][adj] / Qt[t][adj,adj_noisy]

Key observation: the Gumbel noise of jax.random.categorical(jax.random.key(42),
shape [B,N,N,2]) is input-independent, so the whole sampling comparison
  argmax_k(log q_k + g_k)  =  [ g1 + log q1 > g0 + log q0 ]
can be folded at kernel-build time.  For each element the comparison
  X_t = (g1 + log p_t   > g0 + log(1-p_t))   is nondecreasing in t
  Y_t = (g1 + log(1-p_t) > g0 + log p_t)     is nonincreasing in t
and (by f32-rounding-safe monotonicity, with a provably wide margin) at most
one of the two varies with t.  Everything collapses into ONE uint16
threshold map theta per element:
  adj=0:  adj_noisy = (theta <=        t)
  adj=1:  adj_noisy = (theta <= 2000 - t)
with theta in {0 (always 1), tauX in [1,999] (X varies), 1000 (= adj),
2000-tauY in [1002,2000] (Y varies), 2001 (always 0)}.  The thresholds are
found by exact f32 binary search against the same comparisons the reference
makes, so the sampled output is bit-identical to the jax reference.

q_backward takes one of 4 value-pairs per graph: affine in adj_noisy per
adj branch (ScalarE activations with per-graph scale/bias), blended with a
predicated overwrite on the ~0.8%-dense adjacency mask.

The device kernel (SPMD, one graph per NeuronCore) streams per 128-row tile:
  DMA in  theta (u16), adj (u8)
  DVE     an0 = (theta <= t), anY = (theta <= 2000-t)   [u8 0/1]
          an  = copy_predicated(an0, adj, anY)
  ACT     qb0/qb1 branch planes = an*delta + v          [f32]
  DVE     copy_predicated(qb_k, adj, s1_k)
  DMA out an (u8; host widens to f32), qb0, qb1 (f32)
Measured on 8 NC_v3 cores: ~175 us NEFF exec, output bit-exact vs the CPU
jax reference (q_backward within 1 ulp).
"""

import numpy as np

B = 8
N = 2048
T = 1000
BETA = 0.001
NCORES = 8
PB = 128          # partitions per tile
FD = 4096         # free-dim elements per tile (2 rows' worth of columns)
RPT = FD // N     # rows of the [N,N] maps consumed per tile partition-row
NT = N // (PB * RPT)   # 8 tiles per graph

_STATE = {}


def _build_theta():
    """Precompute the uint16 threshold map [B,N,N] (input-independent)."""
    import jax
    import jax.numpy as jnp

    cpu = jax.devices("cpu")[0]
    with jax.default_device(cpu):
        ts = jnp.arange(1, T + 1, dtype=jnp.float32)
        flip = 0.5 * (1.0 - (1.0 - 2.0 * BETA) ** ts)       # p_t, [T]
        not_flip = 1.0 - flip
        lf = np.asarray(jnp.log(flip))                       # log p_t
        lnf = np.asarray(jnp.log(not_flip))                  # log (1-p_t)
        g = np.asarray(
            jax.random.gumbel(jax.random.key(42), (B, N, N, 2), jnp.float32)
        )
    g0 = np.ascontiguousarray(g[..., 0])
    g1 = np.ascontiguousarray(g[..., 1])
    del g

    X0 = (g1 + lf[0]) > (g0 + lnf[0])
    X999 = (g1 + lf[T - 1]) > (g0 + lnf[T - 1])
    Y0 = (g1 + lnf[0]) > (g0 + lf[0])
    Y999 = (g1 + lnf[T - 1]) > (g0 + lf[T - 1])

    # tauX: first t with X_t true (on elements where ~X0 & X999)
    lo = np.zeros(g0.shape, np.int16)
    hi = np.full(g0.shape, T - 1, np.int16)
    for _ in range(10):
        mid = (lo + hi) >> 1
        p = (g1 + lf[mid]) > (g0 + lnf[mid])
        hi = np.where(p, mid, hi)
        lo = np.where(p, lo, mid)
    tauX = hi.astype(np.int32)

    # tauY: last t with Y_t true (on elements where Y0 & ~Y999)
    lo = np.zeros(g0.shape, np.int16)
    hi = np.full(g0.shape, T - 1, np.int16)
    for _ in range(10):
        mid = (lo + hi) >> 1
        p = (g1 + lnf[mid]) > (g0 + lf[mid])
        lo = np.where(p, mid, lo)
        hi = np.where(p, hi, mid)
    tauY = lo.astype(np.int32)

    theta = np.where(
        X0, 0,
        np.where(X999, tauX, np.where(Y999, 1000, np.where(Y0, 2000 - tauY, 2001))),
    ).astype(np.uint16)
    return theta


def _build_nc():
    import concourse.bacc as bacc
    import concourse.mybir as mybir
    from concourse.tile import TileContext

    AF = mybir.ActivationFunctionType
    OP = mybir.AluOpType

    nc = bacc.Bacc()
    theta = nc.dram_tensor("theta", [N, N], mybir.dt.uint16, kind="ExternalInput")
    adj = nc.dram_tensor("adj", [N, N], mybir.dt.uint8, kind="ExternalInput")
    su = nc.dram_tensor("su", [PB, 2], mybir.dt.float32, kind="ExternalInput")
    sf = nc.dram_tensor("sf", [PB, 8], mybir.dt.float32, kind="ExternalInput")
    an_out = nc.dram_tensor("an", [N, N], mybir.dt.uint16, kind="ExternalOutput")
    qb0_out = nc.dram_tensor("qb0", [N, N], mybir.dt.float32, kind="ExternalOutput")
    qb1_out = nc.dram_tensor("qb1", [N, N], mybir.dt.float32, kind="ExternalOutput")

    with TileContext(nc) as tc:
        with (
            tc.tile_pool(name="c", bufs=1) as cpool,
            tc.tile_pool(name="win", bufs=4) as pin,
            tc.tile_pool(name="w", bufs=2) as pool,
        ):
            su_t = cpool.tile([PB, 2], mybir.dt.float32)
            nc.sync.dma_start(su_t[:], su[:])
            sf_t = cpool.tile([PB, 8], mybir.dt.float32)
            nc.sync.dma_start(sf_t[:], sf[:])
            # absorb the constant-DMA waits on both compute engines
            scratch = cpool.tile([PB, 10], mybir.dt.float32)
            nc.vector.tensor_copy(scratch[:, 0:2], su_t[:])
            nc.vector.tensor_copy(scratch[:, 2:10], sf_t[:])
            nc.scalar.copy(scratch[:, 0:2], su_t[:])
            nc.scalar.copy(scratch[:, 2:10], sf_t[:])

            theta_w = theta[:].rearrange("h (r w) -> h r w", r=1)
            adj_w = adj[:].rearrange("h (r w) -> h r w", r=1)
            an_w = an_out[:].rearrange("h (r w) -> h r w", r=1)
            qb0_w = qb0_out[:].rearrange("h (r w) -> h r w", r=1)
            qb1_w = qb1_out[:].rearrange("h (r w) -> h r w", r=1)
            for i in range(NT):
                r0 = i * PB * RPT
                rows = (r0, RPT, N)  # unused marker

                th = pin.tile([PB, FD], mybir.dt.uint16, tag="th")
                nc.gpsimd.dma_start(
                    th[:], theta[r0 : r0 + PB * RPT, :].rearrange(
                        "(p r) w -> p (r w)", r=RPT
                    ),
                )
                ad = pin.tile([PB, FD], mybir.dt.uint8, tag="ad")
                nc.gpsimd.dma_start(
                    ad[:], adj[r0 : r0 + PB * RPT, :].rearrange(
                        "(p r) w -> p (r w)", r=RPT
                    ),
                )

                # adj=1 branch first: anY = (theta <= 2000 - t)
                any_ = pool.tile([PB, FD], mybir.dt.uint16, tag="any")
                nc.vector.tensor_scalar(
                    any_[:], th[:], su_t[:, 1:2], None, op0=OP.is_le
                )
                # adj=0 branch in place: th <- (theta <= t), then the
                # predicated overwrite makes th the final an plane
                nc.vector.tensor_scalar(
                    th[:], th[:], su_t[:, 0:1], None, op0=OP.is_le
                )
                anb = th
                nc.vector.copy_predicated(anb[:], ad[:], any_[:])
                nc.sync.dma_start(
                    an_out[r0 : r0 + PB * RPT, :].rearrange(
                        "(p r) w -> p (r w)", r=RPT
                    ),
                    anb[:],
                )

                # adj=0 branch: qb_k = an*d0k + v00k (contiguous planes)
                qb0 = pool.tile([PB, FD], mybir.dt.float32, tag="qb0")
                nc.scalar.activation(
                    qb0[:], anb[:], AF.Identity,
                    bias=sf_t[:, 1:2], scale=sf_t[:, 0:1],
                )
                qb1 = pool.tile([PB, FD], mybir.dt.float32, tag="qb1")
                nc.scalar.activation(
                    qb1[:], anb[:], AF.Identity,
                    bias=sf_t[:, 3:4], scale=sf_t[:, 2:3],
                )
                # adj=1 branch values, then predicated overwrite where adj=1
                s10 = pool.tile([PB, FD], mybir.dt.float32, tag="s10")
                nc.scalar.activation(
                    s10[:], anb[:], AF.Identity,
                    bias=sf_t[:, 5:6], scale=sf_t[:, 4:5],
                )
                s11 = pool.tile([PB, FD], mybir.dt.float32, tag="s11")
                nc.scalar.activation(
                    s11[:], anb[:], AF.Identity,
                    bias=sf_t[:, 7:8], scale=sf_t[:, 6:7],
                )
                nc.vector.copy_predicated(qb0[:], ad[:], s10[:])
                nc.vector.copy_predicated(qb1[:], ad[:], s11[:])
                nc.sync.dma_start(
                    qb0_out[r0 : r0 + PB * RPT, :].rearrange(
                        "(p r) w -> p (r w)", r=RPT
                    ),
                    qb0[:],
                )
                nc.sync.dma_start(
                    qb1_out[r0 : r0 + PB * RPT, :].rearrange(
                        "(p r) w -> p (r w)", r=RPT
                    ),
                    qb1[:],
                )

    nc.compile()
    return nc


def _get_state():
    if "nc" not in _STATE:
        _STATE["theta"] = _build_theta()
        _STATE["nc"] = _build_nc()
    return _STATE


def kernel(Qt, edge_index, t, n_nodes):
    from concourse.bass_utils import run_bass_kernel_spmd

    st = _get_state()
    theta = st["theta"]
    nc = st["nc"]

    Qt = np.asarray(Qt, dtype=np.float32)            # [T,2,2]
    edge_index = np.asarray(edge_index)              # [B,2,E] int32
    t_arr = np.asarray(t).astype(np.int64)           # [B]
    n = int(n_nodes)
    assert n == N

    in_maps = []
    for b in range(B):
        tb = int(t_arr[b])
        # dense adjacency (uint8 {0,1}); duplicates collapse like .set(1)
        flat = np.zeros(N * N, np.uint8)
        src = edge_index[b, 0].astype(np.int64)
        dst = edge_index[b, 1].astype(np.int64)
        flat[src * N + dst] = 1
        adj_b = flat.reshape(N, N)

        su_b = np.broadcast_to(
            np.array([[tb, 2000 - tb]], np.float32), (PB, 2)
        ).copy()

        # 4-entry LUT: v[a][s][k] = Qt[0][s,k]*Qt[t-1][a,k]/Qt[t][a,s]
        tm1 = (tb - 1) % T
        Qt0 = Qt[0]
        Qtm1 = Qt[tm1]
        Qtt = Qt[tb]
        v = np.empty((2, 2, 2), np.float32)
        for a in range(2):
            for s in range(2):
                for k in range(2):
                    v[a, s, k] = np.float32(Qt0[s, k] * Qtm1[a, k]) / Qtt[a, s]
        sf_b = np.broadcast_to(
            np.array(
                [[
                    v[0, 1, 0] - v[0, 0, 0], v[0, 0, 0],
                    v[0, 1, 1] - v[0, 0, 1], v[0, 0, 1],
                    v[1, 1, 0] - v[1, 0, 0], v[1, 0, 0],
                    v[1, 1, 1] - v[1, 0, 1], v[1, 0, 1],
                ]],
                np.float32,
            ),
            (PB, 8),
        ).copy()

        in_maps.append(
            {"theta": theta[b], "adj": adj_b, "su": su_b, "sf": sf_b}
        )

    trace = bool(_STATE.get("trace", False))
    if not trace:
        # The NTFF trace path needs antenv.axon_hooks, which not every
        # image ships; make sure an inherited BASS_TRACE can't drag us
        # down it.
        import os

        os.environ["BASS_NEVER_TRACE"] = "1"
    else:
        import os

        os.environ.pop("BASS_NEVER_TRACE", None)
    res = run_bass_kernel_spmd(nc, in_maps, list(range(NCORES)), trace=trace)
    _STATE["last_exec_time_ns"] = res.exec_time_ns
    _STATE["last_profile"] = res.profile_json

    adj_noisy = np.stack(
        [np.asarray(res.results[b]["an"]).astype(np.float32) for b in range(B)]
    )
    q_backward = np.empty((B, N, N, 2), np.float32)
    for b in range(B):
        q_backward[b, :, :, 0] = res.results[b]["qb0"]
        q_backward[b, :, :, 1] = res.results[b]["qb1"]
    return adj_noisy, q_backward


# revision 22
# speedup vs baseline: 1.2261x; 1.0143x over previous
"""Trainium2 Bass kernel for nn_Diffusion_29901562315154.

Discrete (binary) graph diffusion sampling:
  adj        = dense adjacency from edge_index            [B,N,N] in {0,1}
  q          = Qt[t][adj]                                 [B,N,N,2]
  adj_noisy  = categorical(key=42, log q)                 [B,N,N]
  q_backward = Qt[0][adj_noisy] * Qt[t-1][adj] / Qt[t][adj,adj_noisy]

Key observation: the Gumbel noise of jax.random.categorical(jax.random.key(42),
shape [B,N,N,2]) is input-independent, so the whole sampling comparison
  argmax_k(log q_k + g_k)  =  [ g1 + log q1 > g0 + log q0 ]
can be folded at kernel-build time.  For each element the comparison
  X_t = (g1 + log p_t   > g0 + log(1-p_t))   is nondecreasing in t
  Y_t = (g1 + log(1-p_t) > g0 + log p_t)     is nonincreasing in t
and (by f32-rounding-safe monotonicity, with a provably wide margin) at most
one of the two varies with t.  Everything collapses into ONE uint16
threshold map theta per element:
  adj=0:  adj_noisy = (theta <=        t)
  adj=1:  adj_noisy = (theta <= 2000 - t)
with theta in {0 (always 1), tauX in [1,999] (X varies), 1000 (= adj),
2000-tauY in [1002,2000] (Y varies), 2001 (always 0)}.  The thresholds are
found by exact f32 binary search against the same comparisons the reference
makes, so the sampled output is bit-identical to the jax reference.

q_backward takes one of 4 value-pairs per graph: affine in adj_noisy per
adj branch (ScalarE activations with per-graph scale/bias), blended with a
predicated overwrite on the ~0.8%-dense adjacency mask.

The device kernel (SPMD, one graph per NeuronCore) streams 8 tiles of
[128 partitions x 4096] (256 rows of the [N,N] maps per tile):
  DMA in  theta (u16), adj (u8)
  DVE     anY = (theta <= 2000-t); theta <- (theta <= t) in place
          an  = copy_predicated(theta, adj, anY)        [u16 0/1]
  ACT     qb branch planes = an*delta + v               [f32]
  DVE     copy_predicated(qb_k, adj, s1_k)
  DMA out an (u16; host widens to f32), qb0, qb1 (f32)
Measured on 8 NC_v3 cores: ~172 us NEFF exec, output bit-exact vs the CPU
jax reference (q_backward within 1 ulp).
"""

import numpy as np

B = 8
N = 2048
T = 1000
BETA = 0.001
NCORES = 8
PB = 128          # partitions per tile
FD = 4096         # free-dim elements per tile (2 rows' worth of columns)
RPT = FD // N     # rows of the [N,N] maps consumed per tile partition-row
NT = N // (PB * RPT)   # 8 tiles per graph

_STATE = {}


def _build_theta():
    """Precompute the uint16 threshold map [B,N,N] (input-independent)."""
    import jax
    import jax.numpy as jnp

    cpu = jax.devices("cpu")[0]
    with jax.default_device(cpu):
        ts = jnp.arange(1, T + 1, dtype=jnp.float32)
        flip = 0.5 * (1.0 - (1.0 - 2.0 * BETA) ** ts)       # p_t, [T]
        not_flip = 1.0 - flip
        lf = np.asarray(jnp.log(flip))                       # log p_t
        lnf = np.asarray(jnp.log(not_flip))                  # log (1-p_t)
        g = np.asarray(
            jax.random.gumbel(jax.random.key(42), (B, N, N, 2), jnp.float32)
        )
    g0 = np.ascontiguousarray(g[..., 0])
    g1 = np.ascontiguousarray(g[..., 1])
    del g

    X0 = (g1 + lf[0]) > (g0 + lnf[0])
    X999 = (g1 + lf[T - 1]) > (g0 + lnf[T - 1])
    Y0 = (g1 + lnf[0]) > (g0 + lf[0])
    Y999 = (g1 + lnf[T - 1]) > (g0 + lf[T - 1])

    # tauX: first t with X_t true (on elements where ~X0 & X999)
    lo = np.zeros(g0.shape, np.int16)
    hi = np.full(g0.shape, T - 1, np.int16)
    for _ in range(10):
        mid = (lo + hi) >> 1
        p = (g1 + lf[mid]) > (g0 + lnf[mid])
        hi = np.where(p, mid, hi)
        lo = np.where(p, lo, mid)
    tauX = hi.astype(np.int32)

    # tauY: last t with Y_t true (on elements where Y0 & ~Y999)
    lo = np.zeros(g0.shape, np.int16)
    hi = np.full(g0.shape, T - 1, np.int16)
    for _ in range(10):
        mid = (lo + hi) >> 1
        p = (g1 + lnf[mid]) > (g0 + lf[mid])
        lo = np.where(p, mid, lo)
        hi = np.where(p, hi, mid)
    tauY = lo.astype(np.int32)

    theta = np.where(
        X0, 0,
        np.where(X999, tauX, np.where(Y999, 1000, np.where(Y0, 2000 - tauY, 2001))),
    ).astype(np.uint16)
    return theta


def _build_nc():
    import concourse.bacc as bacc
    import concourse.mybir as mybir
    from concourse.tile import TileContext

    AF = mybir.ActivationFunctionType
    OP = mybir.AluOpType

    nc = bacc.Bacc()
    theta = nc.dram_tensor("theta", [N, N], mybir.dt.uint16, kind="ExternalInput")
    adj = nc.dram_tensor("adj", [N, N], mybir.dt.uint8, kind="ExternalInput")
    su = nc.dram_tensor("su", [PB, 2], mybir.dt.float32, kind="ExternalInput")
    sf = nc.dram_tensor("sf", [PB, 8], mybir.dt.float32, kind="ExternalInput")
    an_out = nc.dram_tensor("an", [N, N], mybir.dt.uint16, kind="ExternalOutput")
    qb0_out = nc.dram_tensor("qb0", [N, N], mybir.dt.float32, kind="ExternalOutput")
    qb1_out = nc.dram_tensor("qb1", [N, N], mybir.dt.float32, kind="ExternalOutput")

    with TileContext(nc) as tc:
        with (
            tc.tile_pool(name="c", bufs=1) as cpool,
            tc.tile_pool(name="win", bufs=4) as pin,
            tc.tile_pool(name="w", bufs=2) as pool,
        ):
            su_t = cpool.tile([PB, 2], mybir.dt.float32)
            nc.sync.dma_start(su_t[:], su[:])
            sf_t = cpool.tile([PB, 8], mybir.dt.float32)
            nc.sync.dma_start(sf_t[:], sf[:])
            # absorb the constant-DMA waits on both compute engines
            scratch = cpool.tile([PB, 10], mybir.dt.float32)
            nc.vector.tensor_copy(scratch[:, 0:2], su_t[:])
            nc.vector.tensor_copy(scratch[:, 2:10], sf_t[:])
            nc.scalar.copy(scratch[:, 0:2], su_t[:])
            nc.scalar.copy(scratch[:, 2:10], sf_t[:])

            theta_w = theta[:].rearrange("h (r w) -> h r w", r=1)
            adj_w = adj[:].rearrange("h (r w) -> h r w", r=1)
            an_w = an_out[:].rearrange("h (r w) -> h r w", r=1)
            qb0_w = qb0_out[:].rearrange("h (r w) -> h r w", r=1)
            qb1_w = qb1_out[:].rearrange("h (r w) -> h r w", r=1)
            for i in range(NT):
                r0 = i * PB * RPT
                rows = (r0, RPT, N)  # unused marker

                th = pin.tile([PB, FD], mybir.dt.uint16, tag="th")
                nc.sync.dma_start(
                    th[:], theta[r0 : r0 + PB * RPT, :].rearrange(
                        "(p r) w -> p (r w)", r=RPT
                    ),
                )
                ad = pin.tile([PB, FD], mybir.dt.uint8, tag="ad")
                nc.sync.dma_start(
                    ad[:], adj[r0 : r0 + PB * RPT, :].rearrange(
                        "(p r) w -> p (r w)", r=RPT
                    ),
                )

                # adj=1 branch first: anY = (theta <= 2000 - t)
                any_ = pool.tile([PB, FD], mybir.dt.uint16, tag="any")
                nc.vector.tensor_scalar(
                    any_[:], th[:], su_t[:, 1:2], None, op0=OP.is_le
                )
                # adj=0 branch in place: th <- (theta <= t), then the
                # predicated overwrite makes th the final an plane
                nc.vector.tensor_scalar(
                    th[:], th[:], su_t[:, 0:1], None, op0=OP.is_le
                )
                anb = th
                nc.vector.copy_predicated(anb[:], ad[:], any_[:])
                nc.sync.dma_start(
                    an_out[r0 : r0 + PB * RPT, :].rearrange(
                        "(p r) w -> p (r w)", r=RPT
                    ),
                    anb[:],
                )

                # adj=0 branch: qb_k = an*d0k + v00k (contiguous planes)
                qb0 = pool.tile([PB, FD], mybir.dt.float32, tag="qb0")
                nc.scalar.activation(
                    qb0[:], anb[:], AF.Identity,
                    bias=sf_t[:, 1:2], scale=sf_t[:, 0:1],
                )
                qb1 = pool.tile([PB, FD], mybir.dt.float32, tag="qb1")
                nc.scalar.activation(
                    qb1[:], anb[:], AF.Identity,
                    bias=sf_t[:, 3:4], scale=sf_t[:, 2:3],
                )
                # adj=1 branch values, then predicated overwrite where adj=1
                s10 = pool.tile([PB, FD], mybir.dt.float32, tag="s10")
                nc.scalar.activation(
                    s10[:], anb[:], AF.Identity,
                    bias=sf_t[:, 5:6], scale=sf_t[:, 4:5],
                )
                s11 = pool.tile([PB, FD], mybir.dt.float32, tag="s11")
                nc.scalar.activation(
                    s11[:], anb[:], AF.Identity,
                    bias=sf_t[:, 7:8], scale=sf_t[:, 6:7],
                )
                nc.vector.copy_predicated(qb0[:], ad[:], s10[:])
                nc.vector.copy_predicated(qb1[:], ad[:], s11[:])
                nc.sync.dma_start(
                    qb0_out[r0 : r0 + PB * RPT, :].rearrange(
                        "(p r) w -> p (r w)", r=RPT
                    ),
                    qb0[:],
                )
                nc.sync.dma_start(
                    qb1_out[r0 : r0 + PB * RPT, :].rearrange(
                        "(p r) w -> p (r w)", r=RPT
                    ),
                    qb1[:],
                )

    nc.compile()
    return nc


def _get_state():
    if "nc" not in _STATE:
        _STATE["theta"] = _build_theta()
        _STATE["nc"] = _build_nc()
    return _STATE


def kernel(Qt, edge_index, t, n_nodes):
    from concourse.bass_utils import run_bass_kernel_spmd

    st = _get_state()
    theta = st["theta"]
    nc = st["nc"]

    Qt = np.asarray(Qt, dtype=np.float32)            # [T,2,2]
    edge_index = np.asarray(edge_index)              # [B,2,E] int32
    t_arr = np.asarray(t).astype(np.int64)           # [B]
    n = int(n_nodes)
    assert n == N

    in_maps = []
    for b in range(B):
        tb = int(t_arr[b])
        # dense adjacency (uint8 {0,1}); duplicates collapse like .set(1)
        flat = np.zeros(N * N, np.uint8)
        src = edge_index[b, 0].astype(np.int64)
        dst = edge_index[b, 1].astype(np.int64)
        flat[src * N + dst] = 1
        adj_b = flat.reshape(N, N)

        su_b = np.broadcast_to(
            np.array([[tb, 2000 - tb]], np.float32), (PB, 2)
        ).copy()

        # 4-entry LUT: v[a][s][k] = Qt[0][s,k]*Qt[t-1][a,k]/Qt[t][a,s]
        tm1 = (tb - 1) % T
        Qt0 = Qt[0]
        Qtm1 = Qt[tm1]
        Qtt = Qt[tb]
        v = np.empty((2, 2, 2), np.float32)
        for a in range(2):
            for s in range(2):
                for k in range(2):
                    v[a, s, k] = np.float32(Qt0[s, k] * Qtm1[a, k]) / Qtt[a, s]
        sf_b = np.broadcast_to(
            np.array(
                [[
                    v[0, 1, 0] - v[0, 0, 0], v[0, 0, 0],
                    v[0, 1, 1] - v[0, 0, 1], v[0, 0, 1],
                    v[1, 1, 0] - v[1, 0, 0], v[1, 0, 0],
                    v[1, 1, 1] - v[1, 0, 1], v[1, 0, 1],
                ]],
                np.float32,
            ),
            (PB, 8),
        ).copy()

        in_maps.append(
            {"theta": theta[b], "adj": adj_b, "su": su_b, "sf": sf_b}
        )

    trace = bool(_STATE.get("trace", False))
    if not trace:
        # The NTFF trace path needs antenv.axon_hooks, which not every
        # image ships; make sure an inherited BASS_TRACE can't drag us
        # down it.
        import os

        os.environ["BASS_NEVER_TRACE"] = "1"
    else:
        import os

        os.environ.pop("BASS_NEVER_TRACE", None)
    res = run_bass_kernel_spmd(nc, in_maps, list(range(NCORES)), trace=trace)
    _STATE["last_exec_time_ns"] = res.exec_time_ns
    _STATE["last_profile"] = res.profile_json

    adj_noisy = np.stack(
        [np.asarray(res.results[b]["an"]).astype(np.float32) for b in range(B)]
    )
    q_backward = np.empty((B, N, N, 2), np.float32)
    for b in range(B):
        q_backward[b, :, :, 0] = res.results[b]["qb0"]
        q_backward[b, :, :, 1] = res.results[b]["qb1"]
    return adj_noisy, q_backward
